# revision 13
# baseline (speedup 1.0000x reference)
"""CARAFE naive upsampling kernel for Trainium2 (Bass/Tile), 8-core SPMD.

Problem: features [8, 256, 64, 64] f32, masks [8, 25, 128, 128] f32
         -> out [8, 256, 128, 128] f32
    out[n, c, 2h+a, 2w+b] = sum_{di,dj} feat[n, c, h+di-2, w+dj-2]
                                        * mask[n, 5*di+dj, 2h+a, 2w+b]
Sharding: data-parallel over batch dim, one batch per NeuronCore.

Per-core algorithm (tensor-engine formulation):
  - featT[wp, hf, c] = feat[c, hf, wp-2]   (PE transpose; wp in [0,68) zero-padded,
    rows 68:128 zero so matmuls contract K=128 -- K<128 runs at half PE clock)
  - per output row-pair h: banded mask matrix B2[wp, slot] with
    slot = (wq+4)*20 + di*4 + ho_l*2 + b; nonzero at wp in [wq, wq+4] holding
    mask[5di + (wp-wq), 2h+ho_l, 2wq+b].  Built by PE-transposing mask rows
    into mt[wq+4, dj*20+di*4+ho_l*2+b] then ONE diagonal-access-pattern
    SBUF->SBUF DMA per dj per group.  Zero slots are zeroed once (band geometry
    is h-invariant); out-of-range "fictitious" slots absorb edge garbage and
    are never read by the matmuls.
  - out[c, (ho_l, wo)] = sum_di featT[:, h+di-2, c].T @ B2[:, slots | di]
    accumulated over di in PSUM -> <=5 matmuls of [128,128]x[128,256] per ctile.
  - feature loads are chunked per 4-row group and software-pipelined into the
    main loop (2 groups ahead), so matmuls start within a few us of launch.
"""

import sys

if "/opt/trn_rl_repo" not in sys.path:
    sys.path.insert(0, "/opt/trn_rl_repo")

import numpy as np

N_CORES = 8
C = 256
H = 64
W = 64
KK = 5
HO = 128
WO = 128
NI = 25  # 5*5 taps
WP = 68  # padded width positions
B2_FREE = 72 * 20  # slots: (wq+4)*20 + di*4 + ho_l*2 + b, wq in [-4, 68)

_CACHE = {}


def _build_bass(mm_dtype_name="bfloat16"):
    import concourse.bass as bass
    import concourse.bacc as bacc
    import concourse.mybir as mybir
    from concourse import tile
    from concourse.bass import AP

    dt = mybir.dt
    f32 = dt.float32
    mm_dt = getattr(dt, mm_dtype_name)

    nc = bacc.Bacc("TRN2", target_bir_lowering=False, debug=False)
    feat_d = nc.dram_tensor("features", [C, H, W], f32, kind="ExternalInput")
    mask_d = nc.dram_tensor("masks", [NI, HO, WO], f32, kind="ExternalInput")
    out_d = nc.dram_tensor("out", [C, HO, WO], f32, kind="ExternalOutput")

    with tile.TileContext(nc) as tc:
        with (
            tc.tile_pool(name="const", bufs=1) as constp,
            tc.tile_pool(name="featS", bufs=6) as fsp,
            tc.tile_pool(name="featB", bufs=6) as fbp,
            tc.tile_pool(name="featT", bufs=1) as ftp,
            tc.tile_pool(name="maskS", bufs=1) as msp,
            tc.tile_pool(name="b2", bufs=1) as b2p,
            tc.tile_pool(name="mt", bufs=1) as mtp,
            tc.tile_pool(name="mtx", bufs=2) as mtxp,
            tc.tile_pool(name="mtd", bufs=2, space="DRAM") as mtdp,
            tc.tile_pool(name="osb", bufs=1) as osbp,
            tc.tile_pool(name="pmt", bufs=2, space="PSUM") as pmtp,
            tc.tile_pool(name="pmtb", bufs=2, space="PSUM") as pmtbp,
            tc.tile_pool(name="pmm", bufs=4, space="PSUM") as pmmp,
        ):
            # ---- identity matrices for PE transposes ----
            # affine_select runs on gpsimd; route through a DVE copy so PE
            # matmuls only ever wait on {DVE, DMA} sems (ISA limit: 2 waits)
            ident = constp.tile([128, 128], f32)
            identb = constp.tile([128, 128], mm_dt)
            ones = constp.tile([128, 128], f32)
            nc.vector.memset(ones[:], 1.0)
            nc.gpsimd.affine_select(
                ones[:],
                ones[:],
                pattern=[[-1, 128]],
                compare_op=mybir.AluOpType.is_equal,
                fill=0.0,
                base=0,
                channel_multiplier=1,
            )
            nc.vector.tensor_copy(ident[:], ones[:])
            nc.vector.tensor_copy(identb[:], ones[:])

            # ---- load inputs ----
            # whole mask staged once, behind an 8-elem pad: transposes read
            # windows starting 8 elems (4 wq-pairs) early; after group 0 no
            # mask-transpose ever carries a DMA wait. Split into ho-halves
            # [i + 25*(ho//64), (ho%64)*128 + wo] to keep the per-partition
            # pitch under the 64KB ISA stride limit.
            MSP = 64 * WO + 8  # 8200
            maskS = msp.tile([32 + NI, MSP], f32)
            nc.vector.memset(maskS[:, 0:8], 0.0)
            for half in range(2):
                for q in range(4):
                    nc.sync.dma_start(
                        maskS[
                            half * 32 : half * 32 + NI,
                            8 + q * 16 * WO : 8 + (q + 1) * 16 * WO,
                        ],
                        mask_d.ap()[
                            :, half * 64 + q * 16 : half * 64 + (q + 1) * 16, :
                        ].rearrange("i h w -> i (h w)"),
                    )

            # ---- feature transposes: featT[wp, hf, c] ----
            featTs = [
                ftp.tile([128, H // 2, C], mm_dt, tag=f"featT{i}", name=f"featT{i}")
                for i in range(2)
            ]
            # wp rows 64:128 (right pad + K-pad to 128) zeroed once; rows 0:2
            # (left pad) come out zero from the gapped-cast staging below.
            for t in featTs:
                nc.gpsimd.memset(t[64:128].bitcast(f32), 0.0)

            fbs = {}

            def feat_load(g):
                # stage + cast feature rows [4g, 4g+4) x both ctiles.
                # fb is a gapped bf16 staging: each row hl occupies
                # [hl*66, hl*66+66) with 2 leading zeros, so the transpose
                # window picks up correct left-pad zeros at wp 0:2.
                fbs[g] = []
                for ct in range(2):
                    fc = fsp.tile([128, 256], f32, tag="fc", name="fc")
                    src = feat_d.ap()[ct * 128 : (ct + 1) * 128].rearrange(
                        "c h w -> c (h w)"
                    )
                    nc.sync.dma_start(fc[:], src[:, g * 256 : (g + 1) * 256])
                    fb = fbp.tile([128, 264], mm_dt, tag="fb", name="fb")
                    if g < 3:
                        # zero the per-row 2-elem gaps once per rotating buffer
                        for hl in range(4):
                            nc.vector.memset(fb[:, hl * 66 : hl * 66 + 2], 0.0)
                    src_v = AP(
                        tensor=fc.tensor,
                        offset=fc.offset,
                        ap=[[256, 128], [64, 4], [1, 64]],
                    )
                    dst_v = AP(
                        tensor=fb.tensor,
                        offset=fb.offset + 2,
                        ap=[[264, 128], [66, 4], [1, 64]],
                    )
                    nc.vector.tensor_copy(dst_v, src_v)
                    fbs[g].append(fb)

            def feat_transpose(g):
                for ct in range(2):
                    fb = fbs[g][ct]
                    for hl in range(4):
                        hf = 4 * g + hl
                        ptb = pmtbp.tile([128, 128], mm_dt, tag="ptb", name="ptb")
                        nc.tensor.transpose(
                            ptb[0:66, 0:128], fb[:, hl * 66 : hl * 66 + 66], identb[:]
                        )
                        # split PSUM->SBUF feature drains across DVE/ACT
                        if ct == 0:
                            nc.vector.tensor_copy(
                                featTs[hf // 32][
                                    0:66, hf % 32, ct * 128 : (ct + 1) * 128
                                ],
                                ptb[0:66, 0:128],
                            )
                        else:
                            nc.scalar.copy(
                                featTs[hf // 32][
                                    0:66, hf % 32, ct * 128 : (ct + 1) * 128
                                ],
                                ptb[0:66, 0:128],
                            )
                del fbs[g]

            # ---- banded mask buffers, grouped GH row-pairs each, 2 rotating ----
            # v2 slot layout: slot = (di*2 + ho_l)*144 + (wq+4)*2 + b
            # -> matmul rhs for (hl, di) reads TWO CONTIGUOUS 128-elem runs
            #    (strided rhs runs-of-2 cost 270ns/MM vs 157ns contiguous).
            # The diagonal (band) placement is built via a DRAM bounce:
            #   mt --5x shift-dump--> mtD --load--> mt2 --DVE interleave-->
            #   mt3 --dump--> mt3D --one diagonal load (runs of 10)--> B2
            # DRAM-path DMAs spread across all SDMA engines (SBUF->SBUF
            # diagonal scatters pin to SDMA 0-3 and would serialize).
            GH = 4
            B2G = GH * B2_FREE  # 5760
            MTG = GH * 100  # 400
            b2s = [
                b2p.tile([128, B2G], mm_dt, tag=f"b2_{j}", name=f"b2_{j}")
                for j in range(2)
            ]
            nc.gpsimd.memset(b2s[0][:].bitcast(f32), 0.0)
            nc.scalar.memzero(b2s[1][:])
            # mt[wq+4, hl*100 + dj*20 + di*4 + ho_l*2 + b]; partitions [68:72)
            # are fictitious-source garbage, landing only in unread B2 slots
            mts = [
                mtp.tile([72, MTG], mm_dt, tag=f"mt_{j}", name=f"mt_{j}")
                for j in range(2)
            ]
            for t in mts:
                nc.gpsimd.memset(t[64:72].bitcast(f32), 0.0)

            # ---- main loop: software-pipelined one group ahead ----
            # Mask transposes/copies/scatter for group g+1 are emitted BEFORE
            # the matmuls of group g, so the banded-buffer build (PE transposes
            # -> DVE copies -> scatter DMAs) overlaps the previous group's
            # matmuls instead of exposing its latency at every group boundary.
            def build_group(grp):
                g = grp % 2
                mt = mts[g]
                for hl in range(GH):
                    h = grp * GH + hl
                    pm = pmtp.tile([128, 256], f32, tag="pm", name="pm")
                    for ho_l in range(2):
                        for b in range(2):
                            hb = ho_l * 2 + b
                            src = AP(
                                tensor=maskS.tensor,
                                offset=maskS.offset
                                + (h // 32) * 32 * MSP
                                + (2 * (h % 32) + ho_l) * WO
                                + b,
                                ap=[[MSP, NI], [2, WP]],
                            )
                            hb32 = (h // 32) * 32
                            nc.tensor.transpose(
                                pm[0:WP, hb * 25 : (hb + 1) * 25],
                                src,
                                ident[hb32 : hb32 + 25, hb32 : hb32 + 25],
                            )
                    pm_v = AP(
                        tensor=pm.tensor,
                        offset=pm.offset,
                        ap=[[256, WP], [50, 2], [25, 2], [5, 5], [1, 5]],
                    )
                    mt_v = AP(
                        tensor=mt.tensor,
                        offset=mt.offset + hl * 100,
                        ap=[[MTG, WP], [2, 2], [1, 2], [4, 5], [20, 5]],
                    )
                    nc.vector.tensor_copy(mt_v, pm_v)
                # s-shift to DRAM: mtD[wp, hl, s, (di,ho,b)] <- mt[wp+s, hl, dj=4-s]
                mtD = mtdp.tile([68, 400], mm_dt, tag="mtD", name="mtD")
                for sh in range(KK):
                    dst = AP(
                        tensor=mtD.tensor,
                        offset=mtD.offset + sh * 20,
                        ap=[[400, 68], [100, GH], [1, 20]],
                    )
                    src = AP(
                        tensor=mt.tensor,
                        offset=mt.offset + sh * MTG + (4 - sh) * 20,
                        ap=[[MTG, 68], [100, GH], [1, 20]],
                    )
                    nc.sync.dma_start(dst, src)
                # load back, then DVE r-interleave:
                # mt3[wp, hl*100 + (di*2+ho)*10 + 2s+b] <- mt2[wp, hl, s, di, ho, b]
                mt2 = mtxp.tile([68, 400], mm_dt, tag="mt2", name="mt2")
                nc.sync.dma_start(mt2[:], mtD[:])
                mt3 = mtxp.tile([68, 400], mm_dt, tag="mt3", name="mt3")
                for ho in range(2):
                    for b in range(2):
                        dst = AP(
                            tensor=mt3.tensor,
                            offset=mt3.offset + ho * 10 + b,
                            ap=[[400, 68], [100, GH], [20, KK], [2, KK]],
                        )
                        srcv = AP(
                            tensor=mt2.tensor,
                            offset=mt2.offset + ho * 2 + b,
                            ap=[[400, 68], [100, GH], [4, KK], [20, KK]],
                        )
                        nc.vector.tensor_copy(dst, srcv)
                # dump + single diagonal load into the band buffer
                mt3D = mtdp.tile([68, 400], mm_dt, tag="mt3D", name="mt3D")
                nc.sync.dma_start(mt3D[:], mt3[:])
                b2 = b2s[g]
                dst = AP(
                    tensor=b2.tensor,
                    offset=b2.offset,
                    ap=[[B2G + 2, 68], [B2_FREE, GH], [144, 10], [1, 10]],
                )
                src = AP(
                    tensor=mt3D.tensor,
                    offset=mt3D.offset,
                    ap=[[400, 68], [100, GH], [10, 10], [1, 10]],
                )
                nc.sync.dma_start(dst, src)

            def mm_group(grp):
                g = grp % 2
                b2 = b2s[g]
                h_last = grp * GH + GH - 1
                first_pa = True
                osbs = [None, None]
                for hh in range(h_last - GH + 1, h_last + 1):
                    hhl = hh % GH
                    if hhl % 2 == 0:
                        osbs = [
                            osbp.tile(
                                [128, 512], f32, tag=f"osb{ct}", name=f"osb{ct}"
                            )
                            for ct in range(2)
                        ]
                    dis = [di for di in range(KK) if 0 <= hh + di - 2 < H]
                    for ct in range(2):
                        pa = pmmp.tile([128, 256], f32)
                        if first_pa:
                            # dummy PE reads, one per scatter DMA: each absorbs
                            # one DMAHW wait (2-wait Matmult ISA limit). They
                            # write into this pa tile, whose start=True matmul
                            # overwrites them; the WAW pins them before the
                            # matmuls in PE order.
                            nsl = 4 // mybir.dt.size(mm_dt)
                            for j in range(KK):
                                nc.tensor.transpose(
                                    pa[0:1, j : j + 1],
                                    b2[0:1, j * 20 : j * 20 + nsl].bitcast(f32),
                                    ident[0:1, 0:1],
                                )
                            first_pa = False
                        for k, di in enumerate(dis):
                            hf = hh + di - 2
                            # K=128 contraction: rows 68:127 of featT and b2
                            # are zeros. K<128 matmuls run at half PE clock
                            # (HAM stays cold), so padding K to 128 nearly
                            # halves matmul time. rhs reads two contiguous
                            # 128-elem runs (cols (ho_l, wq, b)).
                            rhs = AP(
                                tensor=b2.tensor,
                                offset=b2.offset + hhl * B2_FREE + di * 288 + 8,
                                ap=[[B2G, 128], [144, 2], [1, 128]],
                            )
                            nc.tensor.matmul(
                                pa[:],
                                featTs[hf // 32][
                                    0:128, hf % 32, ct * 128 : (ct + 1) * 128
                                ],
                                rhs,
                                start=(k == 0),
                                stop=(k == len(dis) - 1),
                            )
                        osb = osbs[ct]
                        # split PSUM->SBUF drain across DVE and ACT engines
                        if ct == 0:
                            nc.vector.tensor_copy(
                                osb[:, (hhl % 2) * 256 : (hhl % 2 + 1) * 256], pa[:]
                            )
                        else:
                            nc.scalar.copy(
                                osb[:, (hhl % 2) * 256 : (hhl % 2 + 1) * 256], pa[:]
                            )
                        if hhl % 2 == 1:
                            nc.sync.dma_start(
                                out_d.ap()[
                                    ct * 128 : (ct + 1) * 128,
                                    (hh - 1) * 2 : (hh + 1) * 2,
                                    :,
                                ].rearrange("c h w -> c (h w)"),
                                osb[:],
                            )

            n_groups = H // GH
            feat_load(0)
            build_group(0)
            feat_load(1)
            feat_transpose(0)
            build_group(1)
            feat_load(2)
            feat_transpose(1)
            for grp in range(2, n_groups):
                mm_group(grp - 2)
                if grp + 1 < n_groups:
                    feat_load(grp + 1)
                feat_transpose(grp)
                build_group(grp)
            mm_group(n_groups - 2)
            mm_group(n_groups - 1)
    nc.compile()
    return nc


def _get_nc(mm_dtype_name="bfloat16"):
    key = mm_dtype_name
    if key not in _CACHE:
        _CACHE[key] = _build_bass(mm_dtype_name)
    return _CACHE[key]


def run(features, masks, trace=False, mm_dtype_name="bfloat16"):
    """Returns (out [8,256,128,128] f32, BassKernelResults)."""
    from concourse import bass_utils

    nc = _get_nc(mm_dtype_name)
    features = np.ascontiguousarray(np.asarray(features, dtype=np.float32))
    masks = np.ascontiguousarray(np.asarray(masks, dtype=np.float32))
    in_maps = [{"features": features[i], "masks": masks[i]} for i in range(N_CORES)]
    res = bass_utils.run_bass_kernel_spmd(
        nc, in_maps, core_ids=list(range(N_CORES)), trace=trace
    )
    out = np.stack([res.results[i]["out"] for i in range(N_CORES)], axis=0)
    return out, res


def kernel(features, masks):
    out, _ = run(features, masks, trace=False)
    return out


# revision 15
# speedup vs baseline: 1.0040x; 1.0040x over previous
"""CARAFE naive upsampling kernel for Trainium2 (Bass/Tile), 8-core SPMD.

Problem: features [8, 256, 64, 64] f32, masks [8, 25, 128, 128] f32
         -> out [8, 256, 128, 128] f32
    out[n, c, 2h+a, 2w+b] = sum_{di,dj} feat[n, c, h+di-2, w+dj-2]
                                        * mask[n, 5*di+dj, 2h+a, 2w+b]
Sharding: data-parallel over batch dim, one batch per NeuronCore.

Per-core algorithm (tensor-engine formulation):
  - featT[wp, hf, c] = feat[c, hf, wp-2]   (PE transpose; wp in [0,68) zero-padded,
    rows 68:128 zero so matmuls contract K=128 -- K<128 runs at half PE clock)
  - per output row-pair h: banded mask matrix B2[wp, slot] with
    slot = (wq+4)*20 + di*4 + ho_l*2 + b; nonzero at wp in [wq, wq+4] holding
    mask[5di + (wp-wq), 2h+ho_l, 2wq+b].  Built by PE-transposing mask rows
    into mt[wq+4, dj*20+di*4+ho_l*2+b] then ONE diagonal-access-pattern
    SBUF->SBUF DMA per dj per group.  Zero slots are zeroed once (band geometry
    is h-invariant); out-of-range "fictitious" slots absorb edge garbage and
    are never read by the matmuls.
  - out[c, (ho_l, wo)] = sum_di featT[:, h+di-2, c].T @ B2[:, slots | di]
    accumulated over di in PSUM -> <=5 matmuls of [128,128]x[128,256] per ctile.
  - feature loads are chunked per 4-row group and software-pipelined into the
    main loop (2 groups ahead), so matmuls start within a few us of launch.
"""

import sys

if "/opt/trn_rl_repo" not in sys.path:
    sys.path.insert(0, "/opt/trn_rl_repo")

import numpy as np

N_CORES = 8
C = 256
H = 64
W = 64
KK = 5
HO = 128
WO = 128
NI = 25  # 5*5 taps
WP = 68  # padded width positions
B2_FREE = 10 * 144  # slots: (di*2+ho_l)*144 + (wq+4)*2 + b

_CACHE = {}


def _build_bass(mm_dtype_name="bfloat16"):
    import concourse.bass as bass
    import concourse.bacc as bacc
    import concourse.mybir as mybir
    from concourse import tile
    from concourse.bass import AP

    dt = mybir.dt
    f32 = dt.float32
    mm_dt = getattr(dt, mm_dtype_name)

    nc = bacc.Bacc("TRN2", target_bir_lowering=False, debug=False)
    feat_d = nc.dram_tensor("features", [C, H, W], f32, kind="ExternalInput")
    mask_d = nc.dram_tensor("masks", [NI, HO, WO], f32, kind="ExternalInput")
    out_d = nc.dram_tensor("out", [C, HO, WO], f32, kind="ExternalOutput")

    with tile.TileContext(nc) as tc:
        with (
            tc.tile_pool(name="const", bufs=1) as constp,
            tc.tile_pool(name="featS", bufs=6) as fsp,
            tc.tile_pool(name="featB", bufs=6) as fbp,
            tc.tile_pool(name="featT", bufs=1) as ftp,
            tc.tile_pool(name="maskS", bufs=1) as msp,
            tc.tile_pool(name="b2", bufs=1) as b2p,
            tc.tile_pool(name="mt", bufs=1) as mtp,
            tc.tile_pool(name="mtx", bufs=2) as mtxp,
            tc.tile_pool(name="mtd", bufs=2, space="DRAM") as mtdp,
            tc.tile_pool(name="osb", bufs=1) as osbp,
            tc.tile_pool(name="pmt", bufs=2, space="PSUM") as pmtp,
            tc.tile_pool(name="pmtb", bufs=2, space="PSUM") as pmtbp,
            tc.tile_pool(name="pmm", bufs=4, space="PSUM") as pmmp,
        ):
            # ---- identity matrices for PE transposes ----
            # affine_select runs on gpsimd; route through a DVE copy so PE
            # matmuls only ever wait on {DVE, DMA} sems (ISA limit: 2 waits)
            ident = constp.tile([128, 128], f32)
            identb = constp.tile([128, 128], mm_dt)
            ones = constp.tile([128, 128], f32)
            nc.vector.memset(ones[:], 1.0)
            nc.gpsimd.affine_select(
                ones[:],
                ones[:],
                pattern=[[-1, 128]],
                compare_op=mybir.AluOpType.is_equal,
                fill=0.0,
                base=0,
                channel_multiplier=1,
            )
            nc.vector.tensor_copy(ident[:], ones[:])
            nc.vector.tensor_copy(identb[:], ones[:])

            # ---- load inputs ----
            # whole mask staged once, behind an 8-elem pad: transposes read
            # windows starting 8 elems (4 wq-pairs) early; after group 0 no
            # mask-transpose ever carries a DMA wait. Split into ho-halves
            # [i + 25*(ho//64), (ho%64)*128 + wo] to keep the per-partition
            # pitch under the 64KB ISA stride limit.
            MSP = 64 * WO + 8  # 8200
            maskS = msp.tile([32 + NI, MSP], f32)
            nc.vector.memset(maskS[:, 0:8], 0.0)
            for half in range(2):
                for q in range(4):
                    nc.sync.dma_start(
                        maskS[
                            half * 32 : half * 32 + NI,
                            8 + q * 16 * WO : 8 + (q + 1) * 16 * WO,
                        ],
                        mask_d.ap()[
                            :, half * 64 + q * 16 : half * 64 + (q + 1) * 16, :
                        ].rearrange("i h w -> i (h w)"),
                    )

            # ---- feature transposes: featT[wp, hf, c] ----
            featTs = [
                ftp.tile([128, H // 2, C], mm_dt, tag=f"featT{i}", name=f"featT{i}")
                for i in range(2)
            ]
            # wp rows 64:128 (right pad + K-pad to 128) zeroed once; rows 0:2
            # (left pad) come out zero from the gapped-cast staging below.
            for t in featTs:
                nc.gpsimd.memset(t[64:128].bitcast(f32), 0.0)

            fbs = {}

            def feat_load(g):
                # stage + cast feature rows [4g, 4g+4) x both ctiles.
                # fb is a gapped bf16 staging: each row hl occupies
                # [hl*66, hl*66+66) with 2 leading zeros, so the transpose
                # window picks up correct left-pad zeros at wp 0:2.
                fbs[g] = []
                for ct in range(2):
                    fc = fsp.tile([128, 256], f32, tag="fc", name="fc")
                    src = feat_d.ap()[ct * 128 : (ct + 1) * 128].rearrange(
                        "c h w -> c (h w)"
                    )
                    nc.sync.dma_start(fc[:], src[:, g * 256 : (g + 1) * 256])
                    fb = fbp.tile([128, 264], mm_dt, tag="fb", name="fb")
                    if g < 3:
                        # zero the per-row 2-elem gaps once per rotating buffer
                        for hl in range(4):
                            nc.vector.memset(fb[:, hl * 66 : hl * 66 + 2], 0.0)
                    src_v = AP(
                        tensor=fc.tensor,
                        offset=fc.offset,
                        ap=[[256, 128], [64, 4], [1, 64]],
                    )
                    dst_v = AP(
                        tensor=fb.tensor,
                        offset=fb.offset + 2,
                        ap=[[264, 128], [66, 4], [1, 64]],
                    )
                    nc.vector.tensor_copy(dst_v, src_v)
                    fbs[g].append(fb)

            def feat_transpose(g):
                for ct in range(2):
                    fb = fbs[g][ct]
                    for hl in range(4):
                        hf = 4 * g + hl
                        ptb = pmtbp.tile([128, 128], mm_dt, tag="ptb", name="ptb")
                        nc.tensor.transpose(
                            ptb[0:66, 0:128], fb[:, hl * 66 : hl * 66 + 66], identb[:]
                        )
                        # split PSUM->SBUF feature drains across DVE/ACT
                        if ct == 0:
                            nc.vector.tensor_copy(
                                featTs[hf // 32][
                                    0:66, hf % 32, ct * 128 : (ct + 1) * 128
                                ],
                                ptb[0:66, 0:128],
                            )
                        else:
                            nc.scalar.copy(
                                featTs[hf // 32][
                                    0:66, hf % 32, ct * 128 : (ct + 1) * 128
                                ],
                                ptb[0:66, 0:128],
                            )
                del fbs[g]

            # ---- banded mask buffers, grouped GH row-pairs each, 2 rotating ----
            # v2 slot layout: slot = (di*2 + ho_l)*144 + (wq+4)*2 + b
            # -> matmul rhs for (hl, di) reads TWO CONTIGUOUS 128-elem runs
            #    (strided rhs runs-of-2 cost 270ns/MM vs 157ns contiguous).
            # The diagonal (band) placement is built via a DRAM bounce:
            #   mt --5x shift-dump--> mtD --load--> mt2 --DVE interleave-->
            #   mt3 --dump--> mt3D --one diagonal load (runs of 10)--> B2
            # DRAM-path DMAs spread across all SDMA engines (SBUF->SBUF
            # diagonal scatters pin to SDMA 0-3 and would serialize).
            GH = 4
            B2G = GH * B2_FREE  # 5760
            MTG = GH * 100  # 400
            b2s = [
                b2p.tile([128, B2G], mm_dt, tag=f"b2_{j}", name=f"b2_{j}")
                for j in range(2)
            ]
            nc.gpsimd.memset(b2s[0][:].bitcast(f32), 0.0)
            nc.scalar.memzero(b2s[1][:])
            # mt[wq+4, hl*100 + dj*20 + di*4 + ho_l*2 + b]; partitions [68:72)
            # are fictitious-source garbage, landing only in unread B2 slots
            mts = [
                mtp.tile([128, MTG], mm_dt, tag=f"mt_{j}", name=f"mt_{j}")
                for j in range(2)
            ]
            for t in mts:
                nc.gpsimd.memset(t[64:128].bitcast(f32), 0.0)

            # ---- main loop: software-pipelined one group ahead ----
            # Mask transposes/copies/scatter for group g+1 are emitted BEFORE
            # the matmuls of group g, so the banded-buffer build (PE transposes
            # -> DVE copies -> scatter DMAs) overlaps the previous group's
            # matmuls instead of exposing its latency at every group boundary.
            def build_group(grp):
                g = grp % 2
                mt = mts[g]
                for hl in range(GH):
                    h = grp * GH + hl
                    pm = pmtp.tile([128, 256], f32, tag="pm", name="pm")
                    for ho_l in range(2):
                        for b in range(2):
                            hb = ho_l * 2 + b
                            src = AP(
                                tensor=maskS.tensor,
                                offset=maskS.offset
                                + (h // 32) * 32 * MSP
                                + (2 * (h % 32) + ho_l) * WO
                                + b,
                                ap=[[MSP, NI], [2, WP]],
                            )
                            hb32 = (h // 32) * 32
                            nc.tensor.transpose(
                                pm[0:WP, hb * 25 : (hb + 1) * 25],
                                src,
                                ident[hb32 : hb32 + 25, hb32 : hb32 + 25],
                            )
                    pm_v = AP(
                        tensor=pm.tensor,
                        offset=pm.offset,
                        ap=[[256, WP], [50, 2], [25, 2], [5, 5], [1, 5]],
                    )
                    mt_v = AP(
                        tensor=mt.tensor,
                        offset=mt.offset + hl * 100,
                        ap=[[MTG, WP], [2, 2], [1, 2], [4, 5], [20, 5]],
                    )
                    nc.vector.tensor_copy(mt_v, pm_v)
                # ALL bounce DMAs use 128 partitions: <128-partition dynamic
                # DMAs dispatch to SDMA 0-3 only; 128-partition ones spread
                # across all 16 engines (measured).
                # s-shift to DRAM: mtD[4+wp, hl, s, (di,ho,b)] <- mt[wp+s, hl, dj=4-s]
                # (dst row = src row - s + 4; rows [0,4) and [72,132) absorb junk)
                mtD = mtdp.tile([132, 400], mm_dt, tag="mtD", name="mtD")
                for sh in range(KK):
                    dst = AP(
                        tensor=mtD.tensor,
                        offset=mtD.offset + (4 - sh) * 400 + sh * 20,
                        ap=[[400, 128], [100, GH], [1, 20]],
                    )
                    src = AP(
                        tensor=mt.tensor,
                        offset=mt.offset + (4 - sh) * 20,
                        ap=[[MTG, 128], [100, GH], [1, 20]],
                    )
                    nc.sync.dma_start(dst, src)
                # load back rows [4,132) as wp [0,68)+junk, then DVE r-interleave:
                # mt3[wp, hl*100 + (di*2+ho)*10 + 2s+b] <- mt2[wp, hl, s, di, ho, b]
                mt2 = mtxp.tile([128, 400], mm_dt, tag="mt2", name="mt2")
                srcl = AP(
                    tensor=mtD.tensor,
                    offset=mtD.offset + 4 * 400,
                    ap=[[400, 128], [1, 400]],
                )
                nc.sync.dma_start(mt2[:], srcl)
                mt3 = mtxp.tile([128, 400], mm_dt, tag="mt3", name="mt3")
                for ho in range(2):
                    for b in range(2):
                        dst = AP(
                            tensor=mt3.tensor,
                            offset=mt3.offset + ho * 10 + b,
                            ap=[[400, 68], [100, GH], [20, KK], [2, KK]],
                        )
                        srcv = AP(
                            tensor=mt2.tensor,
                            offset=mt2.offset + ho * 2 + b,
                            ap=[[400, 68], [100, GH], [4, KK], [20, KK]],
                        )
                        nc.vector.tensor_copy(dst, srcv)
                # dump + single diagonal load into the band buffer.
                # The diag MUST be <128 partitions: the 128-partition spread
                # dispatch path recomputes 4-partition chunk bases from the
                # tile pitch, silently breaking flat-slope APs (slope applies
                # only mod 4). The pinned path (SDMA 0-3) handles slopes
                # correctly at ~8ns/descriptor.
                mt3D = mtdp.tile([68, 400], mm_dt, tag="mt3D", name="mt3D")
                nc.sync.dma_start(mt3D[:], mt3[0:68, :])
                b2 = b2s[g]
                dst = AP(
                    tensor=b2.tensor,
                    offset=b2.offset,
                    ap=[[B2G + 2, 68], [B2_FREE, GH], [144, 10], [1, 10]],
                )
                src = AP(
                    tensor=mt3D.tensor,
                    offset=mt3D.offset,
                    ap=[[400, 68], [100, GH], [10, 10], [1, 10]],
                )
                nc.sync.dma_start(dst, src)

            def mm_group(grp):
                g = grp % 2
                b2 = b2s[g]
                h_last = grp * GH + GH - 1
                first_pa = True
                osbs = [None, None]
                for hh in range(h_last - GH + 1, h_last + 1):
                    hhl = hh % GH
                    if hhl % 2 == 0:
                        osbs = [
                            osbp.tile(
                                [128, 512], f32, tag=f"osb{ct}", name=f"osb{ct}"
                            )
                            for ct in range(2)
                        ]
                    dis = [di for di in range(KK) if 0 <= hh + di - 2 < H]
                    for ct in range(2):
                        pa = pmmp.tile([128, 256], f32)
                        if first_pa:
                            # dummy PE reads, one per scatter DMA: each absorbs
                            # one DMAHW wait (2-wait Matmult ISA limit). They
                            # write into this pa tile, whose start=True matmul
                            # overwrites them; the WAW pins them before the
                            # matmuls in PE order.
                            nsl = 4 // mybir.dt.size(mm_dt)
                            for j in range(KK):
                                nc.tensor.transpose(
                                    pa[0:1, j : j + 1],
                                    b2[0:1, j * 20 : j * 20 + nsl].bitcast(f32),
                                    ident[0:1, 0:1],
                                )
                            first_pa = False
                        for k, di in enumerate(dis):
                            hf = hh + di - 2
                            # K=128 contraction: rows 68:127 of featT and b2
                            # are zeros. K<128 matmuls run at half PE clock
                            # (HAM stays cold), so padding K to 128 nearly
                            # halves matmul time. rhs reads two contiguous
                            # 128-elem runs (cols (ho_l, wq, b)).
                            rhs = AP(
                                tensor=b2.tensor,
                                offset=b2.offset + hhl * B2_FREE + di * 288 + 8,
                                ap=[[B2G, 128], [144, 2], [1, 128]],
                            )
                            nc.tensor.matmul(
                                pa[:],
                                featTs[hf // 32][
                                    0:128, hf % 32, ct * 128 : (ct + 1) * 128
                                ],
                                rhs,
                                start=(k == 0),
                                stop=(k == len(dis) - 1),
                            )
                        osb = osbs[ct]
                        # split PSUM->SBUF drain across DVE and ACT engines
                        if ct == 0:
                            nc.vector.tensor_copy(
                                osb[:, (hhl % 2) * 256 : (hhl % 2 + 1) * 256], pa[:]
                            )
                        else:
                            nc.scalar.copy(
                                osb[:, (hhl % 2) * 256 : (hhl % 2 + 1) * 256], pa[:]
                            )
                        if hhl % 2 == 1:
                            nc.sync.dma_start(
                                out_d.ap()[
                                    ct * 128 : (ct + 1) * 128,
                                    (hh - 1) * 2 : (hh + 1) * 2,
                                    :,
                                ].rearrange("c h w -> c (h w)"),
                                osb[:],
                            )

            n_groups = H // GH
            feat_load(0)
            build_group(0)
            feat_load(1)
            feat_transpose(0)
            build_group(1)
            feat_load(2)
            feat_transpose(1)
            for grp in range(2, n_groups):
                mm_group(grp - 2)
                if grp + 1 < n_groups:
                    feat_load(grp + 1)
                feat_transpose(grp)
                build_group(grp)
            mm_group(n_groups - 2)
            mm_group(n_groups - 1)
    nc.compile()
    return nc


def _get_nc(mm_dtype_name="bfloat16"):
    key = mm_dtype_name
    if key not in _CACHE:
        _CACHE[key] = _build_bass(mm_dtype_name)
    return _CACHE[key]


def run(features, masks, trace=False, mm_dtype_name="bfloat16"):
    """Returns (out [8,256,128,128] f32, BassKernelResults)."""
    from concourse import bass_utils

    nc = _get_nc(mm_dtype_name)
    features = np.ascontiguousarray(np.asarray(features, dtype=np.float32))
    masks = np.ascontiguousarray(np.asarray(masks, dtype=np.float32))
    in_maps = [{"features": features[i], "masks": masks[i]} for i in range(N_CORES)]
    res = bass_utils.run_bass_kernel_spmd(
        nc, in_maps, core_ids=list(range(N_CORES)), trace=trace
    )
    out = np.stack([res.results[i]["out"] for i in range(N_CORES)], axis=0)
    return out, res


def kernel(features, masks):
    out, _ = run(features, masks, trace=False)
    return out


# revision 17
# speedup vs baseline: 1.0519x; 1.0478x over previous
"""CARAFE naive upsampling kernel for Trainium2 (Bass/Tile), 8-core SPMD.

Problem: features [8, 256, 64, 64] f32, masks [8, 25, 128, 128] f32
         -> out [8, 256, 128, 128] f32
    out[n, c, 2h+a, 2w+b] = sum_{di,dj} feat[n, c, h+di-2, w+dj-2]
                                        * mask[n, 5*di+dj, 2h+a, 2w+b]
Sharding: data-parallel over batch dim, one batch per NeuronCore.

Per-core algorithm (tensor-engine formulation):
  - featT[wp, hf, c] = feat[c, hf, wp-2]   (PE transpose; wp in [0,68) zero-padded,
    rows 68:128 zero so matmuls contract K=128 -- K<128 runs at half PE clock)
  - per output row-pair h: banded mask matrix B2[wp, slot] with
    slot = (wq+4)*20 + di*4 + ho_l*2 + b; nonzero at wp in [wq, wq+4] holding
    mask[5di + (wp-wq), 2h+ho_l, 2wq+b].  Built by PE-transposing mask rows
    into mt[wq+4, dj*20+di*4+ho_l*2+b] then ONE diagonal-access-pattern
    SBUF->SBUF DMA per dj per group.  Zero slots are zeroed once (band geometry
    is h-invariant); out-of-range "fictitious" slots absorb edge garbage and
    are never read by the matmuls.
  - out[c, (ho_l, wo)] = sum_di featT[:, h+di-2, c].T @ B2[:, slots | di]
    accumulated over di in PSUM -> <=5 matmuls of [128,128]x[128,256] per ctile.
  - feature loads are chunked per 4-row group and software-pipelined into the
    main loop (2 groups ahead), so matmuls start within a few us of launch.
"""

import sys

if "/opt/trn_rl_repo" not in sys.path:
    sys.path.insert(0, "/opt/trn_rl_repo")

import numpy as np

N_CORES = 8
C = 256
H = 64
W = 64
KK = 5
HO = 128
WO = 128
NI = 25  # 5*5 taps
WP = 68  # padded width positions
B2_FREE = 10 * 144  # slots: (di*2+ho_l)*144 + (wq+4)*2 + b

_CACHE = {}


def _build_bass(mm_dtype_name="bfloat16"):
    import concourse.bass as bass
    import concourse.bacc as bacc
    import concourse.mybir as mybir
    from concourse import tile
    from concourse.bass import AP

    dt = mybir.dt
    f32 = dt.float32
    mm_dt = getattr(dt, mm_dtype_name)

    nc = bacc.Bacc("TRN2", target_bir_lowering=False, debug=False)
    feat_d = nc.dram_tensor("features", [C, H, W], f32, kind="ExternalInput")
    mask_d = nc.dram_tensor("masks", [NI, HO, WO], f32, kind="ExternalInput")
    out_d = nc.dram_tensor("out", [C, HO, WO], f32, kind="ExternalOutput")

    with tile.TileContext(nc) as tc:
        with (
            tc.tile_pool(name="const", bufs=1) as constp,
            tc.tile_pool(name="featS", bufs=3) as fsp,
            tc.tile_pool(name="featB", bufs=4) as fbp,
            tc.tile_pool(name="featT", bufs=1) as ftp,
            tc.tile_pool(name="maskS", bufs=1) as msp,
            tc.tile_pool(name="b2", bufs=1) as b2p,
            tc.tile_pool(name="mt", bufs=1) as mtp,
            tc.tile_pool(name="mtx", bufs=3) as mtxp,
            tc.tile_pool(name="mtd", bufs=3, space="DRAM") as mtdp,
            tc.tile_pool(name="osb", bufs=2) as osbp,
            tc.tile_pool(name="pmt", bufs=2, space="PSUM") as pmtp,
            tc.tile_pool(name="pmtb", bufs=2, space="PSUM") as pmtbp,
            tc.tile_pool(name="pmm", bufs=4, space="PSUM") as pmmp,
        ):
            # ---- identity matrices for PE transposes ----
            # affine_select runs on gpsimd; route through a DVE copy so PE
            # matmuls only ever wait on {DVE, DMA} sems (ISA limit: 2 waits)
            ident = constp.tile([128, 128], f32)
            identb = constp.tile([128, 128], mm_dt)
            ones = constp.tile([128, 128], f32)
            nc.vector.memset(ones[:], 1.0)
            nc.gpsimd.affine_select(
                ones[:],
                ones[:],
                pattern=[[-1, 128]],
                compare_op=mybir.AluOpType.is_equal,
                fill=0.0,
                base=0,
                channel_multiplier=1,
            )
            nc.vector.tensor_copy(ident[:], ones[:])
            nc.vector.tensor_copy(identb[:], ones[:])

            # ---- load inputs ----
            # whole mask staged once, behind an 8-elem pad: transposes read
            # windows starting 8 elems (4 wq-pairs) early; after group 0 no
            # mask-transpose ever carries a DMA wait. Split into ho-halves
            # [i + 25*(ho//64), (ho%64)*128 + wo] to keep the per-partition
            # pitch under the 64KB ISA stride limit.
            MSP = 64 * WO + 8  # 8200
            maskS = msp.tile([32 + NI, MSP], f32)
            nc.vector.memset(maskS[:, 0:8], 0.0)
            for half in range(2):
                for q in range(4):
                    nc.sync.dma_start(
                        maskS[
                            half * 32 : half * 32 + NI,
                            8 + q * 16 * WO : 8 + (q + 1) * 16 * WO,
                        ],
                        mask_d.ap()[
                            :, half * 64 + q * 16 : half * 64 + (q + 1) * 16, :
                        ].rearrange("i h w -> i (h w)"),
                    )

            # ---- feature transposes: featT[wp, hf, c] ----
            featTs = [
                ftp.tile([128, H // 2, C], mm_dt, tag=f"featT{i}", name=f"featT{i}")
                for i in range(2)
            ]
            # wp rows 64:128 (right pad + K-pad to 128) zeroed once; rows 0:2
            # (left pad) come out zero from the gapped-cast staging below.
            for t in featTs:
                nc.gpsimd.memset(t[64:128].bitcast(f32), 0.0)

            fbs = {}

            def feat_load(g):
                # stage + cast feature rows [4g, 4g+4) x both ctiles.
                # fb is a gapped bf16 staging: each (ct, row hl) occupies
                # [ct*528 + hl*66, +66) with 2 leading zeros, so the transpose
                # window picks up correct left-pad zeros at wp 0:2.
                fc = fsp.tile([128, 512], f32, tag="fc", name="fc")
                srcf = AP(
                    tensor=feat_d.ap().tensor,
                    offset=g * 256,
                    ap=[[H * W, 128], [128 * H * W, 2], [1, 256]],
                )
                nc.sync.dma_start(fc[:], srcf)
                fb = fbp.tile([128, 1056], mm_dt, tag="fb", name="fb")
                if g < 4:
                    # zero the per-row 2-elem gaps once per rotating buffer
                    for ct in range(2):
                        for hl in range(4):
                            o = ct * 528 + hl * 66
                            nc.vector.memset(fb[:, o : o + 2], 0.0)
                src_v = AP(
                    tensor=fc.tensor,
                    offset=fc.offset,
                    ap=[[512, 128], [256, 2], [64, 4], [1, 64]],
                )
                dst_v = AP(
                    tensor=fb.tensor,
                    offset=fb.offset + 2,
                    ap=[[1056, 128], [528, 2], [66, 4], [1, 64]],
                )
                nc.vector.tensor_copy(dst_v, src_v)
                fbs[g] = fb

            def feat_transpose(g):
                for ct in range(2):
                    fb = fbs[g]
                    for hl in range(4):
                        hf = 4 * g + hl
                        ptb = pmtbp.tile([128, 128], mm_dt, tag="ptb", name="ptb")
                        o = ct * 528 + hl * 66
                        nc.tensor.transpose(
                            ptb[0:66, 0:128], fb[:, o : o + 66], identb[:]
                        )
                        # split PSUM->SBUF feature drains across DVE/ACT
                        if ct == 0:
                            nc.vector.tensor_copy(
                                featTs[hf // 32][
                                    0:66, hf % 32, ct * 128 : (ct + 1) * 128
                                ],
                                ptb[0:66, 0:128],
                            )
                        else:
                            nc.scalar.copy(
                                featTs[hf // 32][
                                    0:66, hf % 32, ct * 128 : (ct + 1) * 128
                                ],
                                ptb[0:66, 0:128],
                            )
                del fbs[g]

            # ---- banded mask buffers, grouped GH row-pairs each, 2 rotating ----
            # v2 slot layout: slot = (di*2 + ho_l)*144 + (wq+4)*2 + b
            # -> matmul rhs for (hl, di) reads TWO CONTIGUOUS 128-elem runs
            #    (strided rhs runs-of-2 cost 270ns/MM vs 157ns contiguous).
            # The diagonal (band) placement is built via a DRAM bounce:
            #   mt --5x shift-dump--> mtD --load--> mt2 --DVE interleave-->
            #   mt3 --dump--> mt3D --one diagonal load (runs of 10)--> B2
            # DRAM-path DMAs spread across all SDMA engines (SBUF->SBUF
            # diagonal scatters pin to SDMA 0-3 and would serialize).
            GH = 4
            B2G = GH * B2_FREE  # 5760
            MTG = GH * 100  # 400
            b2s = [
                b2p.tile([128, B2G], mm_dt, tag=f"b2_{j}", name=f"b2_{j}")
                for j in range(3)
            ]
            nc.gpsimd.memset(b2s[0][:].bitcast(f32), 0.0)
            nc.scalar.memzero(b2s[1][:])
            nc.gpsimd.memset(b2s[2][:].bitcast(f32), 0.0)
            # mt[wq+4, hl*100 + dj*20 + di*4 + ho_l*2 + b]; partitions [68:72)
            # are fictitious-source garbage, landing only in unread B2 slots
            mts = [
                mtp.tile([128, MTG], mm_dt, tag=f"mt_{j}", name=f"mt_{j}")
                for j in range(3)
            ]
            for t in mts:
                nc.gpsimd.memset(t[64:128].bitcast(f32), 0.0)

            # ---- main loop: software-pipelined one group ahead ----
            # Mask transposes/copies/scatter for group g+1 are emitted BEFORE
            # the matmuls of group g, so the banded-buffer build (PE transposes
            # -> DVE copies -> scatter DMAs) overlaps the previous group's
            # matmuls instead of exposing its latency at every group boundary.
            def build_group(grp):
                g = grp % 3
                mt = mts[g]
                for hl in range(GH):
                    h = grp * GH + hl
                    pm = pmtp.tile([128, 256], f32, tag="pm", name="pm")
                    for ho_l in range(2):
                        for b in range(2):
                            hb = ho_l * 2 + b
                            src = AP(
                                tensor=maskS.tensor,
                                offset=maskS.offset
                                + (h // 32) * 32 * MSP
                                + (2 * (h % 32) + ho_l) * WO
                                + b,
                                ap=[[MSP, NI], [2, WP]],
                            )
                            hb32 = (h // 32) * 32
                            nc.tensor.transpose(
                                pm[0:WP, hb * 25 : (hb + 1) * 25],
                                src,
                                ident[hb32 : hb32 + 25, hb32 : hb32 + 25],
                            )
                    pm_v = AP(
                        tensor=pm.tensor,
                        offset=pm.offset,
                        ap=[[256, WP], [50, 2], [25, 2], [5, 5], [1, 5]],
                    )
                    mt_v = AP(
                        tensor=mt.tensor,
                        offset=mt.offset + hl * 100,
                        ap=[[MTG, WP], [2, 2], [1, 2], [4, 5], [20, 5]],
                    )
                    nc.vector.tensor_copy(mt_v, pm_v)
                # ALL bounce DMAs use 128 partitions: <128-partition dynamic
                # DMAs dispatch to SDMA 0-3 only; 128-partition ones spread
                # across all 16 engines (measured).
                # s-shift to DRAM: mtD[4+wp, hl, s, (di,ho,b)] <- mt[wp+s, hl, dj=4-s]
                # (dst row = src row - s + 4; rows [0,4) and [72,132) absorb junk)
                mtD = mtdp.tile([132, 400], mm_dt, tag="mtD", name="mtD")
                for sh in range(KK):
                    dst = AP(
                        tensor=mtD.tensor,
                        offset=mtD.offset + (4 - sh) * 400 + sh * 20,
                        ap=[[400, 128], [100, GH], [1, 20]],
                    )
                    src = AP(
                        tensor=mt.tensor,
                        offset=mt.offset + (4 - sh) * 20,
                        ap=[[MTG, 128], [100, GH], [1, 20]],
                    )
                    nc.scalar.dma_start(dst, src)
                # load back rows [4,132) as wp [0,68)+junk, then DVE r-interleave:
                # mt3[wp, hl*100 + (di*2+ho)*10 + 2s+b] <- mt2[wp, hl, s, di, ho, b]
                mt2 = mtxp.tile([128, 400], mm_dt, tag="mt2", name="mt2")
                srcl = AP(
                    tensor=mtD.tensor,
                    offset=mtD.offset + 4 * 400,
                    ap=[[400, 128], [1, 400]],
                )
                nc.sync.dma_start(mt2[:], srcl)
                mt3 = mtxp.tile([128, 400], mm_dt, tag="mt3", name="mt3")
                for ho in range(2):
                    for b in range(2):
                        dst = AP(
                            tensor=mt3.tensor,
                            offset=mt3.offset + ho * 10 + b,
                            ap=[[400, 68], [100, GH], [20, KK], [2, KK]],
                        )
                        srcv = AP(
                            tensor=mt2.tensor,
                            offset=mt2.offset + ho * 2 + b,
                            ap=[[400, 68], [100, GH], [4, KK], [20, KK]],
                        )
                        nc.vector.tensor_copy(dst, srcv)
                # dump + single diagonal load into the band buffer.
                # The diag MUST be <128 partitions: the 128-partition spread
                # dispatch path recomputes 4-partition chunk bases from the
                # tile pitch, silently breaking flat-slope APs (slope applies
                # only mod 4). The pinned path (SDMA 0-3) handles slopes
                # correctly at ~8ns/descriptor.
                mt3D = mtdp.tile([68, 400], mm_dt, tag="mt3D", name="mt3D")
                nc.scalar.dma_start(mt3D[:], mt3[0:68, :])
                b2 = b2s[g]
                dst = AP(
                    tensor=b2.tensor,
                    offset=b2.offset,
                    ap=[[B2G + 2, 68], [B2_FREE, GH], [144, 10], [1, 10]],
                )
                src = AP(
                    tensor=mt3D.tensor,
                    offset=mt3D.offset,
                    ap=[[400, 68], [100, GH], [10, 10], [1, 10]],
                )
                nc.sync.dma_start(dst, src)

            def mm_group(grp):
                g = grp % 3
                b2 = b2s[g]
                h_last = grp * GH + GH - 1
                first_pa = True
                osbs = [None, None]
                for hh in range(h_last - GH + 1, h_last + 1):
                    hhl = hh % GH
                    if hhl % 2 == 0:
                        osb1 = osbp.tile([128, 1024], f32, tag="osb", name="osb")
                    dis = [di for di in range(KK) if 0 <= hh + di - 2 < H]
                    for ct in range(2):
                        pa = pmmp.tile([128, 256], f32)
                        if first_pa:
                            # dummy PE reads, one per scatter DMA: each absorbs
                            # one DMAHW wait (2-wait Matmult ISA limit). They
                            # write into this pa tile, whose start=True matmul
                            # overwrites them; the WAW pins them before the
                            # matmuls in PE order.
                            nsl = 4 // mybir.dt.size(mm_dt)
                            for j in range(KK):
                                nc.tensor.transpose(
                                    pa[0:1, j : j + 1],
                                    b2[0:1, j * 20 : j * 20 + nsl].bitcast(f32),
                                    ident[0:1, 0:1],
                                )
                            first_pa = False
                        for k, di in enumerate(dis):
                            hf = hh + di - 2
                            # K=128 contraction: rows 68:127 of featT and b2
                            # are zeros. K<128 matmuls run at half PE clock
                            # (HAM stays cold), so padding K to 128 nearly
                            # halves matmul time. rhs reads two contiguous
                            # 128-elem runs (cols (ho_l, wq, b)).
                            rhs = AP(
                                tensor=b2.tensor,
                                offset=b2.offset + hhl * B2_FREE + di * 288 + 8,
                                ap=[[B2G, 128], [144, 2], [1, 128]],
                            )
                            nc.tensor.matmul(
                                pa[:],
                                featTs[hf // 32][
                                    0:128, hf % 32, ct * 128 : (ct + 1) * 128
                                ],
                                rhs,
                                start=(k == 0),
                                stop=(k == len(dis) - 1),
                            )
                        # split PSUM->SBUF drain across DVE and ACT engines;
                        # osb layout [c, ct*512 + (hhl%2)*256 + col]
                        oo = ct * 512 + (hhl % 2) * 256
                        if ct == 0:
                            nc.vector.tensor_copy(osb1[:, oo : oo + 256], pa[:])
                        else:
                            nc.scalar.copy(osb1[:, oo : oo + 256], pa[:])
                        if hhl % 2 == 1 and ct == 1:
                            dsto = AP(
                                tensor=out_d.ap().tensor,
                                offset=(hh - 1) * 2 * WO,
                                ap=[[HO * WO, 128], [128 * HO * WO, 2], [1, 512]],
                            )
                            nc.sync.dma_start(dsto, osb1[:])

            n_groups = H // GH
            feat_load(0)
            build_group(0)
            feat_load(1)
            feat_transpose(0)
            build_group(1)
            feat_load(2)
            feat_transpose(1)
            build_group(2)
            feat_load(3)
            feat_transpose(2)
            for grp in range(3, n_groups):
                mm_group(grp - 3)
                if grp + 1 < n_groups:
                    feat_load(grp + 1)
                feat_transpose(grp)
                build_group(grp)
            mm_group(n_groups - 3)
            mm_group(n_groups - 2)
            mm_group(n_groups - 1)
    nc.compile()
    return nc


def _get_nc(mm_dtype_name="bfloat16"):
    key = mm_dtype_name
    if key not in _CACHE:
        _CACHE[key] = _build_bass(mm_dtype_name)
    return _CACHE[key]


def run(features, masks, trace=False, mm_dtype_name="bfloat16"):
    """Returns (out [8,256,128,128] f32, BassKernelResults)."""
    from concourse import bass_utils

    nc = _get_nc(mm_dtype_name)
    features = np.ascontiguousarray(np.asarray(features, dtype=np.float32))
    masks = np.ascontiguousarray(np.asarray(masks, dtype=np.float32))
    in_maps = [{"features": features[i], "masks": masks[i]} for i in range(N_CORES)]
    res = bass_utils.run_bass_kernel_spmd(
        nc, in_maps, core_ids=list(range(N_CORES)), trace=trace
    )
    out = np.stack([res.results[i]["out"] for i in range(N_CORES)], axis=0)
    return out, res


def kernel(features, masks):
    out, _ = run(features, masks, trace=False)
    return out


# revision 18
# speedup vs baseline: 1.3198x; 1.2546x over previous
"""CARAFE naive upsampling kernel for Trainium2 (Bass/Tile), 8-core SPMD.

Problem: features [8, 256, 64, 64] f32, masks [8, 25, 128, 128] f32
         -> out [8, 256, 128, 128] f32
    out[n, c, 2h+a, 2w+b] = sum_{di,dj} feat[n, c, h+di-2, w+dj-2]
                                        * mask[n, 5*di+dj, 2h+a, 2w+b]
Sharding: data-parallel over batch dim, one batch per NeuronCore.

Per-core algorithm (tensor-engine formulation):
  - featT[wp, hf, c] = feat[c, hf, wp-2]   (PE transpose; wp in [0,68) zero-padded,
    rows 68:128 zero so matmuls contract K=128 -- K<128 runs at half PE clock)
  - per output row-pair h: banded mask matrix B2[wp, slot] with
    slot = (wq+4)*20 + di*4 + ho_l*2 + b; nonzero at wp in [wq, wq+4] holding
    mask[5di + (wp-wq), 2h+ho_l, 2wq+b].  Built by PE-transposing mask rows
    into mt[wq+4, dj*20+di*4+ho_l*2+b] then ONE diagonal-access-pattern
    SBUF->SBUF DMA per dj per group.  Zero slots are zeroed once (band geometry
    is h-invariant); out-of-range "fictitious" slots absorb edge garbage and
    are never read by the matmuls.
  - out[c, (ho_l, wo)] = sum_di featT[:, h+di-2, c].T @ B2[:, slots | di]
    accumulated over di in PSUM -> <=5 matmuls of [128,128]x[128,256] per ctile.
  - feature loads are chunked per 4-row group and software-pipelined into the
    main loop (2 groups ahead), so matmuls start within a few us of launch.
"""

import sys

if "/opt/trn_rl_repo" not in sys.path:
    sys.path.insert(0, "/opt/trn_rl_repo")

import numpy as np

N_CORES = 8
C = 256
H = 64
W = 64
KK = 5
HO = 128
WO = 128
NI = 25  # 5*5 taps
WP = 68  # padded width positions
B2_FREE = 72 * 20  # slots: (wq+4)*20 + di*4 + ho_l*2 + b, wq in [-4, 68)

_CACHE = {}


def _build_bass(mm_dtype_name="bfloat16"):
    import concourse.bass as bass
    import concourse.bacc as bacc
    import concourse.mybir as mybir
    from concourse import tile
    from concourse.bass import AP

    dt = mybir.dt
    f32 = dt.float32
    mm_dt = getattr(dt, mm_dtype_name)

    nc = bacc.Bacc("TRN2", target_bir_lowering=False, debug=False)
    feat_d = nc.dram_tensor("features", [C, H, W], f32, kind="ExternalInput")
    mask_d = nc.dram_tensor("masks", [NI, HO, WO], f32, kind="ExternalInput")
    out_d = nc.dram_tensor("out", [C, HO, WO], f32, kind="ExternalOutput")

    with tile.TileContext(nc) as tc:
        with (
            tc.tile_pool(name="const", bufs=1) as constp,
            tc.tile_pool(name="featS", bufs=3) as fsp,
            tc.tile_pool(name="featB", bufs=4) as fbp,
            tc.tile_pool(name="featT", bufs=1) as ftp,
            tc.tile_pool(name="maskS", bufs=1) as msp,
            tc.tile_pool(name="b2", bufs=1) as b2p,
            tc.tile_pool(name="mt", bufs=1) as mtp,
            tc.tile_pool(name="osb", bufs=2) as osbp,
            tc.tile_pool(name="pmt", bufs=2, space="PSUM") as pmtp,
            tc.tile_pool(name="pmtb", bufs=2, space="PSUM") as pmtbp,
            tc.tile_pool(name="pmm", bufs=4, space="PSUM") as pmmp,
        ):
            # ---- identity matrices for PE transposes ----
            # affine_select runs on gpsimd; route through a DVE copy so PE
            # matmuls only ever wait on {DVE, DMA} sems (ISA limit: 2 waits)
            ident = constp.tile([128, 128], f32)
            identb = constp.tile([128, 128], mm_dt)
            ones = constp.tile([128, 128], f32)
            nc.vector.memset(ones[:], 1.0)
            nc.gpsimd.affine_select(
                ones[:],
                ones[:],
                pattern=[[-1, 128]],
                compare_op=mybir.AluOpType.is_equal,
                fill=0.0,
                base=0,
                channel_multiplier=1,
            )
            nc.vector.tensor_copy(ident[:], ones[:])
            nc.vector.tensor_copy(identb[:], ones[:])

            # ---- load inputs ----
            # whole mask staged once, behind an 8-elem pad: transposes read
            # windows starting 8 elems (4 wq-pairs) early; after group 0 no
            # mask-transpose ever carries a DMA wait. Split into ho-halves
            # [i + 25*(ho//64), (ho%64)*128 + wo] to keep the per-partition
            # pitch under the 64KB ISA stride limit.
            MSP = 64 * WO + 8  # 8200
            maskS = msp.tile([32 + NI, MSP], f32)
            nc.vector.memset(maskS[:, 0:8], 0.0)
            for half in range(2):
                for q in range(4):
                    nc.sync.dma_start(
                        maskS[
                            half * 32 : half * 32 + NI,
                            8 + q * 16 * WO : 8 + (q + 1) * 16 * WO,
                        ],
                        mask_d.ap()[
                            :, half * 64 + q * 16 : half * 64 + (q + 1) * 16, :
                        ].rearrange("i h w -> i (h w)"),
                    )

            # ---- feature transposes: featT[wp, hf, c] ----
            featTs = [
                ftp.tile([128, H // 2, C], mm_dt, tag=f"featT{i}", name=f"featT{i}")
                for i in range(2)
            ]
            # wp rows 64:128 (right pad + K-pad to 128) zeroed once; rows 0:2
            # (left pad) come out zero from the gapped-cast staging below.
            for t in featTs:
                nc.gpsimd.memset(t[64:128].bitcast(f32), 0.0)

            fbs = {}

            def feat_load(g):
                # stage + cast feature rows [4g, 4g+4) x both ctiles.
                # fb is a gapped bf16 staging: each (ct, row hl) occupies
                # [ct*528 + hl*66, +66) with 2 leading zeros, so the transpose
                # window picks up correct left-pad zeros at wp 0:2.
                fc = fsp.tile([128, 512], f32, tag="fc", name="fc")
                srcf = AP(
                    tensor=feat_d.ap().tensor,
                    offset=g * 256,
                    ap=[[H * W, 128], [128 * H * W, 2], [1, 256]],
                )
                nc.sync.dma_start(fc[:], srcf)
                fb = fbp.tile([128, 1056], mm_dt, tag="fb", name="fb")
                if g < 4:
                    # zero the per-row 2-elem gaps once per rotating buffer
                    for ct in range(2):
                        for hl in range(4):
                            o = ct * 528 + hl * 66
                            nc.vector.memset(fb[:, o : o + 2], 0.0)
                src_v = AP(
                    tensor=fc.tensor,
                    offset=fc.offset,
                    ap=[[512, 128], [256, 2], [64, 4], [1, 64]],
                )
                dst_v = AP(
                    tensor=fb.tensor,
                    offset=fb.offset + 2,
                    ap=[[1056, 128], [528, 2], [66, 4], [1, 64]],
                )
                nc.vector.tensor_copy(dst_v, src_v)
                fbs[g] = fb

            def feat_transpose(g):
                for ct in range(2):
                    fb = fbs[g]
                    for hl in range(4):
                        hf = 4 * g + hl
                        ptb = pmtbp.tile([128, 128], mm_dt, tag="ptb", name="ptb")
                        o = ct * 528 + hl * 66
                        nc.tensor.transpose(
                            ptb[0:66, 0:128], fb[:, o : o + 66], identb[:]
                        )
                        # split PSUM->SBUF feature drains across DVE/ACT
                        if ct == 0:
                            nc.vector.tensor_copy(
                                featTs[hf // 32][
                                    0:66, hf % 32, ct * 128 : (ct + 1) * 128
                                ],
                                ptb[0:66, 0:128],
                            )
                        else:
                            nc.scalar.copy(
                                featTs[hf // 32][
                                    0:66, hf % 32, ct * 128 : (ct + 1) * 128
                                ],
                                ptb[0:66, 0:128],
                            )
                del fbs[g]

            # ---- banded mask buffers, grouped GH row-pairs each, 2 rotating ----
            # v2 slot layout: slot = (di*2 + ho_l)*144 + (wq+4)*2 + b
            # -> matmul rhs for (hl, di) reads TWO CONTIGUOUS 128-elem runs
            #    (strided rhs runs-of-2 cost 270ns/MM vs 157ns contiguous).
            # The diagonal (band) placement is built via a DRAM bounce:
            #   mt --5x shift-dump--> mtD --load--> mt2 --DVE interleave-->
            #   mt3 --dump--> mt3D --one diagonal load (runs of 10)--> B2
            # DRAM-path DMAs spread across all SDMA engines (SBUF->SBUF
            # diagonal scatters pin to SDMA 0-3 and would serialize).
            GH = 4
            B2G = GH * B2_FREE  # 5760
            MTG = GH * 100  # 400
            b2s = [
                b2p.tile([128, B2G], mm_dt, tag=f"b2_{j}", name=f"b2_{j}")
                for j in range(2)
            ]
            nc.gpsimd.memset(b2s[0][:].bitcast(f32), 0.0)
            nc.scalar.memzero(b2s[1][:])
            # mt[wq+4, hl*100 + dj*20 + di*4 + ho_l*2 + b]; partitions [68:72)
            # are fictitious-source garbage, landing only in unread B2 slots
            mts = [
                mtp.tile([128, MTG], mm_dt, tag=f"mt_{j}", name=f"mt_{j}")
                for j in range(2)
            ]
            for t in mts:
                nc.gpsimd.memset(t[64:128].bitcast(f32), 0.0)

            # ---- main loop: software-pipelined one group ahead ----
            # Mask transposes/copies/scatter for group g+1 are emitted BEFORE
            # the matmuls of group g, so the banded-buffer build (PE transposes
            # -> DVE copies -> scatter DMAs) overlaps the previous group's
            # matmuls instead of exposing its latency at every group boundary.
            def build_group(grp):
                g = grp % 2
                mt = mts[g]
                for hl in range(GH):
                    h = grp * GH + hl
                    pm = pmtp.tile([128, 256], f32, tag="pm", name="pm")
                    for ho_l in range(2):
                        for b in range(2):
                            hb = ho_l * 2 + b
                            src = AP(
                                tensor=maskS.tensor,
                                offset=maskS.offset
                                + (h // 32) * 32 * MSP
                                + (2 * (h % 32) + ho_l) * WO
                                + b,
                                ap=[[MSP, NI], [2, WP]],
                            )
                            hb32 = (h // 32) * 32
                            nc.tensor.transpose(
                                pm[0:WP, hb * 25 : (hb + 1) * 25],
                                src,
                                ident[hb32 : hb32 + 25, hb32 : hb32 + 25],
                            )
                    pm_v = AP(
                        tensor=pm.tensor,
                        offset=pm.offset,
                        ap=[[256, WP], [50, 2], [25, 2], [5, 5], [1, 5]],
                    )
                    mt_v = AP(
                        tensor=mt.tensor,
                        offset=mt.offset + hl * 100,
                        ap=[[MTG, WP], [2, 2], [1, 2], [4, 5], [20, 5]],
                    )
                    nc.vector.tensor_copy(mt_v, pm_v)
                b2 = b2s[g]
                for j in range(KK):
                    dst = AP(
                        tensor=b2.tensor,
                        offset=b2.offset + j * 20,
                        ap=[[B2G + 20, WP], [B2_FREE, GH], [4, 5], [2, 2], [1, 2]],
                    )
                    src = AP(
                        tensor=mt.tensor,
                        offset=mt.offset + 80 + j * (MTG - 20),
                        ap=[[MTG, WP], [100, GH], [4, 5], [2, 2], [1, 2]],
                    )
                    nc.sync.dma_start(dst, src)

            def mm_group(grp):
                g = grp % 2
                b2 = b2s[g]
                h_last = grp * GH + GH - 1
                first_pa = True
                osbs = [None, None]
                for hh in range(h_last - GH + 1, h_last + 1):
                    hhl = hh % GH
                    if hhl % 2 == 0:
                        osb1 = osbp.tile([128, 1024], f32, tag="osb", name="osb")
                    dis = [di for di in range(KK) if 0 <= hh + di - 2 < H]
                    for ct in range(2):
                        pa = pmmp.tile([128, 256], f32)
                        if first_pa:
                            # dummy PE reads, one per scatter DMA: each absorbs
                            # one DMAHW wait (2-wait Matmult ISA limit). They
                            # write into this pa tile, whose start=True matmul
                            # overwrites them; the WAW pins them before the
                            # matmuls in PE order.
                            nsl = 4 // mybir.dt.size(mm_dt)
                            for j in range(KK):
                                nc.tensor.transpose(
                                    pa[0:1, j : j + 1],
                                    b2[0:1, j * 20 : j * 20 + nsl].bitcast(f32),
                                    ident[0:1, 0:1],
                                )
                            first_pa = False
                        for k, di in enumerate(dis):
                            hf = hh + di - 2
                            # K=128 contraction: rows 68:127 of featT and b2
                            # are zeros. K<128 matmuls run at half PE clock
                            # (HAM stays cold), so padding K to 128 nearly
                            # halves matmul time. rhs reads two contiguous
                            # 128-elem runs (cols (ho_l, wq, b)).
                            rhs = AP(
                                tensor=b2.tensor,
                                offset=b2.offset + hhl * B2_FREE + 80 + di * 4,
                                ap=[[B2G, 128], [2, 2], [20, 64], [1, 2]],
                            )
                            nc.tensor.matmul(
                                pa[:],
                                featTs[hf // 32][
                                    0:128, hf % 32, ct * 128 : (ct + 1) * 128
                                ],
                                rhs,
                                start=(k == 0),
                                stop=(k == len(dis) - 1),
                            )
                        # split PSUM->SBUF drain across DVE and ACT engines;
                        # osb layout [c, ct*512 + (hhl%2)*256 + col]
                        oo = ct * 512 + (hhl % 2) * 256
                        if ct == 0:
                            nc.vector.tensor_copy(osb1[:, oo : oo + 256], pa[:])
                        else:
                            nc.scalar.copy(osb1[:, oo : oo + 256], pa[:])
                        if hhl % 2 == 1 and ct == 1:
                            dsto = AP(
                                tensor=out_d.ap().tensor,
                                offset=(hh - 1) * 2 * WO,
                                ap=[[HO * WO, 128], [128 * HO * WO, 2], [1, 512]],
                            )
                            nc.sync.dma_start(dsto, osb1[:])

            n_groups = H // GH
            feat_load(0)
            build_group(0)
            feat_load(1)
            feat_transpose(0)
            build_group(1)
            feat_load(2)
            feat_transpose(1)
            for grp in range(2, n_groups):
                mm_group(grp - 2)
                if grp + 1 < n_groups:
                    feat_load(grp + 1)
                feat_transpose(grp)
                build_group(grp)
            mm_group(n_groups - 2)
            mm_group(n_groups - 1)
    nc.compile()
    return nc


def _get_nc(mm_dtype_name="bfloat16"):
    key = mm_dtype_name
    if key not in _CACHE:
        _CACHE[key] = _build_bass(mm_dtype_name)
    return _CACHE[key]


def run(features, masks, trace=False, mm_dtype_name="bfloat16"):
    """Returns (out [8,256,128,128] f32, BassKernelResults)."""
    from concourse import bass_utils

    nc = _get_nc(mm_dtype_name)
    features = np.ascontiguousarray(np.asarray(features, dtype=np.float32))
    masks = np.ascontiguousarray(np.asarray(masks, dtype=np.float32))
    in_maps = [{"features": features[i], "masks": masks[i]} for i in range(N_CORES)]
    res = bass_utils.run_bass_kernel_spmd(
        nc, in_maps, core_ids=list(range(N_CORES)), trace=trace
    )
    out = np.stack([res.results[i]["out"] for i in range(N_CORES)], axis=0)
    return out, res


def kernel(features, masks):
    out, _ = run(features, masks, trace=False)
    return out


# revision 19
# speedup vs baseline: 1.3496x; 1.0226x over previous
"""CARAFE naive upsampling kernel for Trainium2 (Bass/Tile), 8-core SPMD.

Problem: features [8, 256, 64, 64] f32, masks [8, 25, 128, 128] f32
         -> out [8, 256, 128, 128] f32
    out[n, c, 2h+a, 2w+b] = sum_{di,dj} feat[n, c, h+di-2, w+dj-2]
                                        * mask[n, 5*di+dj, 2h+a, 2w+b]
Sharding: data-parallel over batch dim, one batch per NeuronCore.

Per-core algorithm (tensor-engine formulation):
  - featT[wp, hf, c] = feat[c, hf, wp-2]   (PE transpose; wp in [0,68) zero-padded,
    rows 68:128 zero so matmuls contract K=128 -- K<128 runs at half PE clock)
  - per output row-pair h: banded mask matrix B2[wp, slot] with
    slot = (wq+4)*20 + di*4 + ho_l*2 + b; nonzero at wp in [wq, wq+4] holding
    mask[5di + (wp-wq), 2h+ho_l, 2wq+b].  Built by PE-transposing mask rows
    into mt[wq+4, dj*20+di*4+ho_l*2+b] then ONE diagonal-access-pattern
    SBUF->SBUF DMA per dj per group.  Zero slots are zeroed once (band geometry
    is h-invariant); out-of-range "fictitious" slots absorb edge garbage and
    are never read by the matmuls.
  - out[c, (ho_l, wo)] = sum_di featT[:, h+di-2, c].T @ B2[:, slots | di]
    accumulated over di in PSUM -> <=5 matmuls of [128,128]x[128,256] per ctile.
  - feature loads are chunked per 4-row group and software-pipelined into the
    main loop (2 groups ahead), so matmuls start within a few us of launch.
"""

import sys

if "/opt/trn_rl_repo" not in sys.path:
    sys.path.insert(0, "/opt/trn_rl_repo")

import numpy as np

N_CORES = 8
C = 256
H = 64
W = 64
KK = 5
HO = 128
WO = 128
NI = 25  # 5*5 taps
WP = 68  # padded width positions
B2_FREE = 72 * 20  # slots: (wq+4)*20 + di*4 + ho_l*2 + b, wq in [-4, 68)

_CACHE = {}


def _build_bass(mm_dtype_name="bfloat16"):
    import concourse.bass as bass
    import concourse.bacc as bacc
    import concourse.mybir as mybir
    from concourse import tile
    from concourse.bass import AP

    dt = mybir.dt
    f32 = dt.float32
    mm_dt = getattr(dt, mm_dtype_name)

    nc = bacc.Bacc("TRN2", target_bir_lowering=False, debug=False)
    feat_d = nc.dram_tensor("features", [C, H, W], f32, kind="ExternalInput")
    mask_d = nc.dram_tensor("masks", [NI, HO, WO], f32, kind="ExternalInput")
    out_d = nc.dram_tensor("out", [C, HO, WO], f32, kind="ExternalOutput")

    with tile.TileContext(nc) as tc:
        with (
            tc.tile_pool(name="const", bufs=1) as constp,
            tc.tile_pool(name="featS", bufs=3) as fsp,
            tc.tile_pool(name="featB", bufs=4) as fbp,
            tc.tile_pool(name="featT", bufs=1) as ftp,
            tc.tile_pool(name="maskS", bufs=1) as msp,
            tc.tile_pool(name="b2", bufs=1) as b2p,
            tc.tile_pool(name="mt", bufs=1) as mtp,
            tc.tile_pool(name="osb", bufs=2) as osbp,
            tc.tile_pool(name="pmt", bufs=2, space="PSUM") as pmtp,
            tc.tile_pool(name="pmtb", bufs=2, space="PSUM") as pmtbp,
            tc.tile_pool(name="pmm", bufs=4, space="PSUM") as pmmp,
        ):
            # ---- identity matrices for PE transposes ----
            # affine_select runs on gpsimd; route through a DVE copy so PE
            # matmuls only ever wait on {DVE, DMA} sems (ISA limit: 2 waits)
            ident = constp.tile([128, 128], f32)
            identb = constp.tile([128, 128], mm_dt)
            ones = constp.tile([128, 128], f32)
            nc.vector.memset(ones[:], 1.0)
            nc.gpsimd.affine_select(
                ones[:],
                ones[:],
                pattern=[[-1, 128]],
                compare_op=mybir.AluOpType.is_equal,
                fill=0.0,
                base=0,
                channel_multiplier=1,
            )
            nc.vector.tensor_copy(ident[:], ones[:])
            nc.vector.tensor_copy(identb[:], ones[:])

            # ---- load inputs ----
            # whole mask staged once, behind an 8-elem pad: transposes read
            # windows starting 8 elems (4 wq-pairs) early; after group 0 no
            # mask-transpose ever carries a DMA wait. Split into ho-halves
            # [i + 25*(ho//64), (ho%64)*128 + wo] to keep the per-partition
            # pitch under the 64KB ISA stride limit.
            MSP = 64 * WO + 8  # 8200
            maskS = msp.tile([32 + NI, MSP], f32)
            nc.vector.memset(maskS[:, 0:8], 0.0)

            def mask_load(half, nq=4):
                for q in range(nq):
                    nc.sync.dma_start(
                        maskS[
                            half * 32 : half * 32 + NI,
                            8 + q * (64 // nq) * WO : 8 + (q + 1) * (64 // nq) * WO,
                        ],
                        mask_d.ap()[
                            :,
                            half * 64 + q * (64 // nq) : half * 64
                            + (q + 1) * (64 // nq),
                            :,
                        ].rearrange("i h w -> i (h w)"),
                    )

            mask_load(0)

            # ---- feature transposes: featT[wp, hf, c] ----
            featTs = [
                ftp.tile([128, H // 2, C], mm_dt, tag=f"featT{i}", name=f"featT{i}")
                for i in range(2)
            ]
            # wp rows 64:128 (right pad + K-pad to 128) zeroed once; rows 0:2
            # (left pad) come out zero from the gapped-cast staging below.
            for t in featTs:
                nc.gpsimd.memset(t[64:128].bitcast(f32), 0.0)

            fbs = {}

            def feat_load(g):
                # stage + cast feature rows [4g, 4g+4) x both ctiles.
                # fb is a gapped bf16 staging: each (ct, row hl) occupies
                # [ct*528 + hl*66, +66) with 2 leading zeros, so the transpose
                # window picks up correct left-pad zeros at wp 0:2.
                fc = fsp.tile([128, 512], f32, tag="fc", name="fc")
                srcf = AP(
                    tensor=feat_d.ap().tensor,
                    offset=g * 256,
                    ap=[[H * W, 128], [128 * H * W, 2], [1, 256]],
                )
                nc.sync.dma_start(fc[:], srcf)
                fb = fbp.tile([128, 1056], mm_dt, tag="fb", name="fb")
                if g < 4:
                    # zero the per-row 2-elem gaps once per rotating buffer
                    for ct in range(2):
                        for hl in range(4):
                            o = ct * 528 + hl * 66
                            nc.vector.memset(fb[:, o : o + 2], 0.0)
                src_v = AP(
                    tensor=fc.tensor,
                    offset=fc.offset,
                    ap=[[512, 128], [256, 2], [64, 4], [1, 64]],
                )
                dst_v = AP(
                    tensor=fb.tensor,
                    offset=fb.offset + 2,
                    ap=[[1056, 128], [528, 2], [66, 4], [1, 64]],
                )
                nc.vector.tensor_copy(dst_v, src_v)
                fbs[g] = fb

            def feat_transpose(g):
                for ct in range(2):
                    fb = fbs[g]
                    for hl in range(4):
                        hf = 4 * g + hl
                        ptb = pmtbp.tile([128, 128], mm_dt, tag="ptb", name="ptb")
                        o = ct * 528 + hl * 66
                        nc.tensor.transpose(
                            ptb[0:66, 0:128], fb[:, o : o + 66], identb[:]
                        )
                        # split PSUM->SBUF feature drains across DVE/ACT
                        if ct == 0:
                            nc.vector.tensor_copy(
                                featTs[hf // 32][
                                    0:66, hf % 32, ct * 128 : (ct + 1) * 128
                                ],
                                ptb[0:66, 0:128],
                            )
                        else:
                            nc.scalar.copy(
                                featTs[hf // 32][
                                    0:66, hf % 32, ct * 128 : (ct + 1) * 128
                                ],
                                ptb[0:66, 0:128],
                            )
                del fbs[g]

            # ---- banded mask buffers, grouped GH row-pairs each, 2 rotating ----
            # v2 slot layout: slot = (di*2 + ho_l)*144 + (wq+4)*2 + b
            # -> matmul rhs for (hl, di) reads TWO CONTIGUOUS 128-elem runs
            #    (strided rhs runs-of-2 cost 270ns/MM vs 157ns contiguous).
            # The diagonal (band) placement is built via a DRAM bounce:
            #   mt --5x shift-dump--> mtD --load--> mt2 --DVE interleave-->
            #   mt3 --dump--> mt3D --one diagonal load (runs of 10)--> B2
            # DRAM-path DMAs spread across all SDMA engines (SBUF->SBUF
            # diagonal scatters pin to SDMA 0-3 and would serialize).
            GH = 4
            B2G = GH * B2_FREE  # 5760
            MTG = GH * 100  # 400
            b2s = [
                b2p.tile([128, B2G], mm_dt, tag=f"b2_{j}", name=f"b2_{j}")
                for j in range(3)
            ]
            nc.gpsimd.memset(b2s[0][:].bitcast(f32), 0.0)
            nc.scalar.memzero(b2s[1][:])
            nc.gpsimd.memset(b2s[2][:].bitcast(f32), 0.0)
            # mt[wq+4, hl*100 + dj*20 + di*4 + ho_l*2 + b]; partitions [68:72)
            # are fictitious-source garbage, landing only in unread B2 slots
            mts = [
                mtp.tile([128, MTG], mm_dt, tag=f"mt_{j}", name=f"mt_{j}")
                for j in range(3)
            ]
            for t in mts:
                nc.gpsimd.memset(t[64:128].bitcast(f32), 0.0)

            # ---- main loop: software-pipelined one group ahead ----
            # Mask transposes/copies/scatter for group g+1 are emitted BEFORE
            # the matmuls of group g, so the banded-buffer build (PE transposes
            # -> DVE copies -> scatter DMAs) overlaps the previous group's
            # matmuls instead of exposing its latency at every group boundary.
            def build_group(grp):
                g = grp % 3
                mt = mts[g]
                for hl in range(GH):
                    h = grp * GH + hl
                    pm = pmtp.tile([128, 256], f32, tag="pm", name="pm")
                    for ho_l in range(2):
                        for b in range(2):
                            hb = ho_l * 2 + b
                            src = AP(
                                tensor=maskS.tensor,
                                offset=maskS.offset
                                + (h // 32) * 32 * MSP
                                + (2 * (h % 32) + ho_l) * WO
                                + b,
                                ap=[[MSP, NI], [2, WP]],
                            )
                            hb32 = (h // 32) * 32
                            nc.tensor.transpose(
                                pm[0:WP, hb * 25 : (hb + 1) * 25],
                                src,
                                ident[hb32 : hb32 + 25, hb32 : hb32 + 25],
                            )
                    pm_v = AP(
                        tensor=pm.tensor,
                        offset=pm.offset,
                        ap=[[256, WP], [50, 2], [25, 2], [5, 5], [1, 5]],
                    )
                    mt_v = AP(
                        tensor=mt.tensor,
                        offset=mt.offset + hl * 100,
                        ap=[[MTG, WP], [2, 2], [1, 2], [4, 5], [20, 5]],
                    )
                    nc.vector.tensor_copy(mt_v, pm_v)
                b2 = b2s[g]
                for j in range(KK):
                    dst = AP(
                        tensor=b2.tensor,
                        offset=b2.offset + j * 20,
                        ap=[[B2G + 20, WP], [B2_FREE, GH], [4, 5], [2, 2], [1, 2]],
                    )
                    src = AP(
                        tensor=mt.tensor,
                        offset=mt.offset + 80 + j * (MTG - 20),
                        ap=[[MTG, WP], [100, GH], [4, 5], [2, 2], [1, 2]],
                    )
                    nc.sync.dma_start(dst, src)

            def mm_group(grp):
                g = grp % 3
                b2 = b2s[g]
                h_last = grp * GH + GH - 1
                first_pa = True
                osbs = [None, None]
                for hh in range(h_last - GH + 1, h_last + 1):
                    hhl = hh % GH
                    if hhl % 2 == 0:
                        osb1 = osbp.tile([128, 1024], f32, tag="osb", name="osb")
                    dis = [di for di in range(KK) if 0 <= hh + di - 2 < H]
                    for ct in range(2):
                        pa = pmmp.tile([128, 256], f32)
                        if first_pa:
                            # dummy PE reads, one per scatter DMA: each absorbs
                            # one DMAHW wait (2-wait Matmult ISA limit). They
                            # write into this pa tile, whose start=True matmul
                            # overwrites them; the WAW pins them before the
                            # matmuls in PE order.
                            nsl = 4 // mybir.dt.size(mm_dt)
                            for j in range(KK):
                                nc.tensor.transpose(
                                    pa[0:1, j : j + 1],
                                    b2[0:1, j * 20 : j * 20 + nsl].bitcast(f32),
                                    ident[0:1, 0:1],
                                )
                            first_pa = False
                        for k, di in enumerate(dis):
                            hf = hh + di - 2
                            # K=128 contraction: rows 68:127 of featT and b2
                            # are zeros. K<128 matmuls run at half PE clock
                            # (HAM stays cold), so padding K to 128 nearly
                            # halves matmul time. rhs reads two contiguous
                            # 128-elem runs (cols (ho_l, wq, b)).
                            rhs = AP(
                                tensor=b2.tensor,
                                offset=b2.offset + hhl * B2_FREE + 80 + di * 4,
                                ap=[[B2G, 128], [2, 2], [20, 64], [1, 2]],
                            )
                            nc.tensor.matmul(
                                pa[:],
                                featTs[hf // 32][
                                    0:128, hf % 32, ct * 128 : (ct + 1) * 128
                                ],
                                rhs,
                                start=(k == 0),
                                stop=(k == len(dis) - 1),
                            )
                        # split PSUM->SBUF drain across DVE and ACT engines;
                        # osb layout [c, ct*512 + (hhl%2)*256 + col]
                        oo = ct * 512 + (hhl % 2) * 256
                        if ct == 0:
                            nc.vector.tensor_copy(osb1[:, oo : oo + 256], pa[:])
                        else:
                            nc.scalar.copy(osb1[:, oo : oo + 256], pa[:])
                        if hhl % 2 == 1 and ct == 1:
                            dsto = AP(
                                tensor=out_d.ap().tensor,
                                offset=(hh - 1) * 2 * WO,
                                ap=[[HO * WO, 128], [128 * HO * WO, 2], [1, 512]],
                            )
                            nc.sync.dma_start(dsto, osb1[:])

            n_groups = H // GH
            feat_load(0)
            build_group(0)
            feat_load(1)
            feat_transpose(0)
            build_group(1)
            feat_load(2)
            feat_transpose(1)
            build_group(2)
            feat_load(3)
            feat_transpose(2)
            for grp in range(3, n_groups):
                mm_group(grp - 3)
                if grp == 4:
                    mask_load(1)
                if grp + 1 < n_groups:
                    feat_load(grp + 1)
                feat_transpose(grp)
                build_group(grp)
            mm_group(n_groups - 3)
            mm_group(n_groups - 2)
            mm_group(n_groups - 1)
    nc.compile()
    return nc


def _get_nc(mm_dtype_name="bfloat16"):
    key = mm_dtype_name
    if key not in _CACHE:
        _CACHE[key] = _build_bass(mm_dtype_name)
    return _CACHE[key]


def run(features, masks, trace=False, mm_dtype_name="bfloat16"):
    """Returns (out [8,256,128,128] f32, BassKernelResults)."""
    from concourse import bass_utils

    nc = _get_nc(mm_dtype_name)
    features = np.ascontiguousarray(np.asarray(features, dtype=np.float32))
    masks = np.ascontiguousarray(np.asarray(masks, dtype=np.float32))
    in_maps = [{"features": features[i], "masks": masks[i]} for i in range(N_CORES)]
    res = bass_utils.run_bass_kernel_spmd(
        nc, in_maps, core_ids=list(range(N_CORES)), trace=trace
    )
    out = np.stack([res.results[i]["out"] for i in range(N_CORES)], axis=0)
    return out, res


def kernel(features, masks):
    out, _ = run(features, masks, trace=False)
    return out
